# revision 3
# baseline (speedup 1.0000x reference)
"""nn_Attention_6373731467473 — linear attention w/ head expansion + LePE.

Full-input contract: kernel(**inputs) takes unsharded inputs, returns full
output. Data-parallel over batch: 8 batch elements -> 8 NeuronCores, no
collectives. Per core, everything runs in bf16 matmuls with fp32 PSUM
accumulation (tolerance is 2e-2 relative).

Pipeline per core (batch element b):
  P1: stream xT tiles; qT = (x @ w_q)^T   [qcol, n] resident SBUF
      k,v row-major per 128-row chunk; softmax(k) over head_dim;
      ktv[h] = softmax(k)_h^T @ v_h accumulated over n.
  P2: assemble block-diag expanded-ktv lhsT tiles (DMA SBUF->SBUF),
      scale 1/sqrt(64) folded in.
  P3: per 1024-col tile: attn^T chunks via block-diag matmuls on qT /
      rolled qT; LePE depthwise 3x3 conv added in-place on DVE as 9
      per-partition-scaled shifted accumulations; proj matmuls into y.

Host prep: transpose+cast x to bf16 (768, 4096) per batch, split w_kv,
fold b_lepe @ w_proj + b_proj into one bias, LePE taps as (1536, 9).
"""

import hashlib
import os
import pickle
from contextlib import ExitStack

import numpy as np

B, N, DIM = 8, 4096, 768
HEADS, HD = 12, 64
EXP = 2
EDIM = EXP * DIM  # 1536
CC = DIM // 128   # 6 contraction chunks
EC = EDIM // 128  # 12 expanded chunks
NT1 = 512         # phase-1 n-tile
NT3 = 1024        # phase-3 n-tile
SCALE = HD ** -0.5

_CACHE_DIR = os.environ.get("BASS_NEFF_DISK_CACHE", "/root/.cache/bass_neff_cache")


def _install_cc_cache():
    """Disk-cache the HLO->NEFF compile (walrus is the slow step)."""
    try:
        import libneuronxla
        from concourse import bass2jax

        bass2jax.install_neuronx_cc_hook()
        if getattr(libneuronxla, "_ant_disk_cache_installed", False):
            return
        inner = libneuronxla.neuronx_cc

        def cached_cc(code, code_format, platform_version, file_prefix):
            try:
                key = hashlib.sha256(
                    b"v1|" + bytes(code) + b"|" + bytes(code_format)
                ).hexdigest()
                path = os.path.join(_CACHE_DIR, key + ".pkl")
                if os.path.exists(path):
                    with open(path, "rb") as f:
                        return pickle.load(f)
            except Exception:
                path = None
            r = inner(code, code_format, platform_version, file_prefix)
            if path is not None:
                try:
                    os.makedirs(_CACHE_DIR, exist_ok=True)
                    tmp = path + f".tmp{os.getpid()}"
                    with open(tmp, "wb") as f:
                        pickle.dump(r, f)
                    os.replace(tmp, path)
                except Exception:
                    pass
            return r

        libneuronxla.neuronx_cc = cached_cc
        libneuronxla._ant_disk_cache_installed = True
    except Exception:
        pass


def _build_nc():
    import concourse.bacc as bacc
    import concourse.mybir as mybir
    import concourse.tile as tile

    f32 = mybir.dt.float32
    bf16 = mybir.dt.bfloat16
    AX = mybir.AxisListType
    OP = mybir.AluOpType
    AF = mybir.ActivationFunctionType

    nc = bacc.Bacc("TRN2", target_bir_lowering=False, debug=False, num_devices=B)

    xt_d = nc.dram_tensor("xt", [DIM, N], bf16, kind="ExternalInput").ap()
    wq_d = nc.dram_tensor("wq", [DIM, DIM], bf16, kind="ExternalInput").ap()
    wk_d = nc.dram_tensor("wk", [DIM, DIM], bf16, kind="ExternalInput").ap()
    wv_d = nc.dram_tensor("wv", [DIM, DIM], bf16, kind="ExternalInput").ap()
    wp_d = nc.dram_tensor("wp", [EDIM, DIM], bf16, kind="ExternalInput").ap()
    taps_d = nc.dram_tensor("taps", [EDIM, 9], f32, kind="ExternalInput").ap()
    bias_d = nc.dram_tensor("bias", [128, DIM], f32, kind="ExternalInput").ap()
    y_d = nc.dram_tensor("y", [N, DIM], f32, kind="ExternalOutput").ap()

    with tile.TileContext(nc) as tc, ExitStack() as ctx:
        persist = ctx.enter_context(tc.tile_pool(name="persist", bufs=1))
        qT = persist.tile([128, CC, N], bf16)           # q^T, chunk-major
        wp_sb = persist.tile([128, EC, DIM], bf16)
        taps_sb = persist.tile([128, EC, 9], f32)
        bias_sb = persist.tile([128, DIM], f32)
        ektv_sb = persist.tile([128, EC, 128], bf16)    # block-diag lhsT per pair
        ktv_acc = persist.tile([64, HEADS * HD], f32)   # ktv accumulator
        ktv_sb = persist.tile([64, HEADS * HD], bf16)   # scaled bf16 ktv

        nc.sync.dma_start(wp_sb, wp_d.rearrange("(t p) m -> p t m", p=128))
        nc.sync.dma_start(taps_sb, taps_d.rearrange("(t p) s -> p t s", p=128))
        nc.sync.dma_start(bias_sb, bias_d)

        # ---------------- Phase 1: qT, k/v, softmax, ktv ----------------
        with tc.tile_pool(name="p1", bufs=3) as p1, \
             tc.tile_pool(name="p1w", bufs=1) as p1w, \
             tc.tile_pool(name="ps_q", bufs=2, space="PSUM") as ps_q, \
             tc.tile_pool(name="ps_kv", bufs=1, space="PSUM") as ps_kv, \
             tc.tile_pool(name="ps_ktv", bufs=1, space="PSUM") as ps_ktv:
            wq_sb = p1w.tile([128, CC, DIM], bf16)
            wk_sb = p1w.tile([128, CC, DIM], bf16)
            wv_sb = p1w.tile([128, CC, DIM], bf16)
            nc.sync.dma_start(wq_sb, wq_d.rearrange("(t p) m -> p t m", p=128))
            nc.sync.dma_start(wk_sb, wk_d.rearrange("(t p) m -> p t m", p=128))
            nc.sync.dma_start(wv_sb, wv_d.rearrange("(t p) m -> p t m", p=128))

            xt_r = xt_d.rearrange("(c p) n -> p c n", p=128)
            for it in range(N // NT1):
                n0 = it * NT1
                xt_sb = p1.tile([128, CC, NT1], bf16, tag="xt")
                nc.sync.dma_start(xt_sb, xt_r[:, :, n0:n0 + NT1])

                # qT chunks
                for t in range(CC):
                    q_ps = ps_q.tile([128, NT1], f32, tag="q")
                    for cc in range(CC):
                        nc.tensor.matmul(
                            q_ps, wq_sb[:, cc, 128 * t:128 * (t + 1)],
                            xt_sb[:, cc, :],
                            start=(cc == 0), stop=(cc == CC - 1),
                        )
                    nc.scalar.copy(out=qT[:, t, n0:n0 + NT1], in_=q_ps)

                # k/v rows, softmax over head_dim, ktv accumulation
                ktv_ps = ps_ktv.tile([64, HEADS * HD], f32, tag="ktv")
                for sub in range(NT1 // 128):
                    k_ps = ps_kv.tile([128, DIM], f32, tag="k")
                    v_ps = ps_kv.tile([128, DIM], f32, tag="v")
                    for cc in range(CC):
                        lhs = xt_sb[:, cc, 128 * sub:128 * (sub + 1)]
                        st, sp = (cc == 0), (cc == CC - 1)
                        nc.tensor.matmul(k_ps[:, 0:512], lhs, wk_sb[:, cc, 0:512],
                                         start=st, stop=sp)
                        nc.tensor.matmul(k_ps[:, 512:768], lhs, wk_sb[:, cc, 512:768],
                                         start=st, stop=sp)
                        nc.tensor.matmul(v_ps[:, 0:512], lhs, wv_sb[:, cc, 0:512],
                                         start=st, stop=sp)
                        nc.tensor.matmul(v_ps[:, 512:768], lhs, wv_sb[:, cc, 512:768],
                                         start=st, stop=sp)

                    exp_sb = p1.tile([128, HEADS, HD], f32, tag="exp")
                    nc.scalar.activation(out=exp_sb.rearrange("p h d -> p (h d)"),
                                         in_=k_ps, func=AF.Exp)
                    ssum = p1.tile([128, HEADS], f32, tag="ssum")
                    nc.vector.reduce_sum(ssum, exp_sb, axis=AX.X)
                    rec = p1.tile([128, HEADS], f32, tag="rec")
                    nc.vector.reciprocal(rec, ssum)
                    ks_bf = p1.tile([128, HEADS, HD], bf16, tag="ks")
                    nc.vector.tensor_tensor(
                        ks_bf, exp_sb,
                        rec[:, :, None].broadcast_to([128, HEADS, HD]), OP.mult)
                    v_bf = p1.tile([128, DIM], bf16, tag="vb")
                    nc.scalar.copy(out=v_bf, in_=v_ps)

                    for h in range(HEADS):
                        nc.tensor.matmul(
                            ktv_ps[:, HD * h:HD * (h + 1)],
                            ks_bf[:, h, :], v_bf[:, HD * h:HD * (h + 1)],
                            start=(sub == 0 and h % 8 == 0),
                            stop=(sub == NT1 // 128 - 1 and h in (7, 11)),
                            skip_group_check=True,
                        )
                # fold this tile's ktv into the fp32 SBUF accumulator
                if it == 0:
                    nc.vector.tensor_copy(out=ktv_acc, in_=ktv_ps)
                else:
                    nc.vector.tensor_tensor(ktv_acc, ktv_acc, ktv_ps, OP.add)

            # scale into bf16 (attention scale folded into ektv)
            nc.scalar.mul(out=ktv_sb, in_=ktv_acc, mul=SCALE)

        # ---------------- Phase 2: block-diag expanded ktv ----------------
        nc.vector.memset(ektv_sb, 0.0)
        for p in range(6):  # non-rolled pairs: heads 2p, 2p+1
            h0, h1 = 2 * p, 2 * p + 1
            nc.sync.dma_start(ektv_sb[0:64, p, 0:64],
                              ktv_sb[:, HD * h0:HD * (h0 + 1)])
            nc.sync.dma_start(ektv_sb[64:128, p, 64:128],
                              ktv_sb[:, HD * h1:HD * (h1 + 1)])
        for r in range(6):  # rolled pairs p=6+r: expanded heads 12+2r, 13+2r
            p = 6 + r
            h, h2 = 2 * r, 2 * r + 1
            h3 = (h2 + 1) % HEADS
            nc.sync.dma_start(ektv_sb[0:64, p, 0:32],
                              ktv_sb[:, HD * h + 32:HD * (h + 1)])
            nc.sync.dma_start(ektv_sb[0:64, p, 32:64],
                              ktv_sb[:, HD * h2:HD * h2 + 32])
            nc.sync.dma_start(ektv_sb[64:128, p, 64:96],
                              ktv_sb[:, HD * h2 + 32:HD * (h2 + 1)])
            nc.sync.dma_start(ektv_sb[64:128, p, 96:128],
                              ktv_sb[:, HD * h3:HD * h3 + 32])

        # ---------------- Phase 3: attn + LePE + proj ----------------
        TAPS = [(dy, dx) for dy in (-1, 0, 1) for dx in (-1, 0, 1)]
        with tc.tile_pool(name="p3", bufs=2) as p3, \
             tc.tile_pool(name="ps_at", bufs=2, space="PSUM") as ps_at, \
             tc.tile_pool(name="ps_y", bufs=2, space="PSUM") as ps_y:
            for it in range(N // NT3):
                n0 = it * NT3
                rows = NT3 // 64          # image rows in this tile
                y0 = n0 // 64             # first global image row
                # rolled-q stream tile with 64-halo on both sides
                a = max(0, n0 - 64)
                b = min(N, n0 + NT3 + 64)
                off = a - (n0 - 64)
                qtr = p3.tile([128, CC, NT3 + 128], bf16, tag="qtr")
                for t in range(CC):
                    nc.sync.dma_start(qtr[0:96, t, off:off + (b - a)],
                                      qT[32:128, t, a:b])
                    nc.sync.dma_start(qtr[96:128, t, off:off + (b - a)],
                                      qT[0:32, (t + 1) % CC, a:b])

                mt = p3.tile([128, EC, NT3], bf16, tag="mt")
                for p in range(EC):
                    for half in range(NT3 // 512):
                        at_ps = ps_at.tile([128, 512], f32, tag="at")
                        if p < 6:
                            rhs = qT[:, p, n0 + 512 * half:n0 + 512 * (half + 1)]
                        else:
                            rhs = qtr[:, p - 6,
                                      64 + 512 * half:64 + 512 * (half + 1)]
                        nc.tensor.matmul(at_ps, ektv_sb[:, p, :], rhs,
                                         start=True, stop=True)
                        nc.scalar.copy(out=mt[:, p, 512 * half:512 * (half + 1)],
                                       in_=at_ps)

                    # LePE: 9 shifted per-partition-scaled accumulations
                    out3 = mt[:, p, :].rearrange("p (y x) -> p y x", x=64)
                    if p < 6:
                        src3 = qT[:, p, :].rearrange("p (y x) -> p y x", x=64)
                        row_of = y0  # src row index = global row
                    else:
                        src3 = qtr[:, p - 6, :].rearrange("p (y x) -> p y x", x=64)
                        row_of = -1  # src row index = local row - 1 offset
                    for k, (dy, dx) in enumerate(TAPS):
                        r0 = max(0, -(y0 + dy))
                        r1 = rows - max(0, y0 + rows - 1 + dy - 63)
                        if dx == 1:
                            xo, xi = (0, 63), (1, 64)
                        elif dx == -1:
                            xo, xi = (1, 64), (0, 63)
                        else:
                            xo, xi = (0, 64), (0, 64)
                        if p < 6:
                            s0 = y0 + r0 + dy
                            s1 = y0 + r1 + dy
                        else:
                            s0 = r0 + dy + 1
                            s1 = r1 + dy + 1
                        o_ap = out3[:, r0:r1, xo[0]:xo[1]]
                        i_ap = src3[:, s0:s1, xi[0]:xi[1]]
                        nc.vector.scalar_tensor_tensor(
                            out=o_ap, in0=i_ap, scalar=taps_sb[:, p, k:k + 1],
                            in1=o_ap, op0=OP.mult, op1=OP.add)

                # proj
                for sub in range(NT3 // 128):
                    y_ps = ps_y.tile([128, DIM], f32, tag="y")
                    for e in range(EC):
                        lhs = mt[:, e, 128 * sub:128 * (sub + 1)]
                        st, sp = (e == 0), (e == EC - 1)
                        nc.tensor.matmul(y_ps[:, 0:512], lhs, wp_sb[:, e, 0:512],
                                         start=st, stop=sp)
                        nc.tensor.matmul(y_ps[:, 512:768], lhs, wp_sb[:, e, 512:768],
                                         start=st, stop=sp)
                    y_sb = p3.tile([128, DIM], f32, tag="ysb")
                    nc.vector.tensor_tensor(y_sb, y_ps, bias_sb, OP.add)
                    nc.sync.dma_start(
                        y_d[n0 + 128 * sub:n0 + 128 * (sub + 1), :], y_sb)

    nc.compile()
    return nc


_nc_cache = None


def kernel(x, w_q, w_kv, w_proj, b_proj, w_lepe, b_lepe):
    global _nc_cache
    import ml_dtypes

    _install_cc_cache()
    from concourse.bass_utils import run_bass_kernel_spmd

    bf = ml_dtypes.bfloat16
    x = np.asarray(x, np.float32)
    w_q = np.asarray(w_q, np.float32)
    w_kv = np.asarray(w_kv, np.float32)
    w_proj = np.asarray(w_proj, np.float32)
    b_proj = np.asarray(b_proj, np.float32)
    w_lepe = np.asarray(w_lepe, np.float32)
    b_lepe = np.asarray(b_lepe, np.float32)

    xt = np.ascontiguousarray(x.transpose(0, 2, 1)).astype(bf)  # (B, DIM, N)
    wq = np.ascontiguousarray(w_q).astype(bf)
    wk = np.ascontiguousarray(w_kv[:, :DIM]).astype(bf)
    wv = np.ascontiguousarray(w_kv[:, DIM:]).astype(bf)
    wp = np.ascontiguousarray(w_proj).astype(bf)
    taps = np.ascontiguousarray(w_lepe.reshape(EDIM, 9)).astype(np.float32)
    bias = (b_proj.astype(np.float64)
            + b_lepe.astype(np.float64) @ w_proj.astype(np.float64)
            ).astype(np.float32)
    bias128 = np.ascontiguousarray(np.broadcast_to(bias, (128, DIM)))

    if _nc_cache is None:
        _nc_cache = _build_nc()
    nc = _nc_cache

    shared = {"wq": wq, "wk": wk, "wv": wv, "wp": wp,
              "taps": taps, "bias": bias128}
    in_maps = [dict(shared, xt=np.ascontiguousarray(xt[bb])) for bb in range(B)]
    res = run_bass_kernel_spmd(nc, in_maps, core_ids=list(range(B)))
    y = np.stack([res.results[bb]["y"] for bb in range(B)], axis=0)
    return y.astype(np.float32)
